# revision 1
# baseline (speedup 1.0000x reference)
"""AttentionProtoNet pooling kernel for 8x TRN2 NeuronCores.

reference (per sample of B=64, L=512, H=768):
    upsilon = tanh(hs @ W_fc.T + b_fc)        [L, H]
    nu      = upsilon @ W_nu                  [L]
    alphas  = softmax(nu)                     [L]
    pooled  = alphas @ hs                     [H]

Strategy: data-parallel over B (8 samples per core), everything on the wire
in fp16 (1 cycle/row on the PE like bf16, but with 10 mantissa bits, and a
single X^T copy feeds both the TensorEngine matmul and the VectorEngine
pooling). The PE runs back-to-back 512-row fp16 matmuls at its 216 ns
roofline cadence; each sample's nu/softmax/pooling epilogue is emitted
inside the NEXT sample's matmul stream so the PE never waits on ACT/DVE,
and the output drain (PE transpose -> copy -> DRAM) trails two samples
behind. DMA goes through the sync HW queue ordered so the PE starts as
early as possible (k0/k1 weights, first sample in halves, k2, remaining
weights, remaining samples). The last sample is processed in two 256-token
halves so most of its softmax/pooling chain overlaps its own matmuls.
"""

import sys

sys.path.insert(0, "/opt/trn_rl_repo")

import numpy as np

B, L, H = 64, 512, 768
NCORES = 8
SPC = B // NCORES            # samples per core
TOK = SPC * L                # tokens per core
HC = H // 128                # 128-partition chunks of H
HL = L // 2                  # token half for the last sample
WARMUP_MM = 9                # junk matmuls: p-state + HAM ramp during DMA

_compiled = {}


def _build():
    import concourse.bass as bass
    import concourse.bacc as bacc
    import concourse.tile as tile
    from concourse import mybir
    from concourse.masks import make_identity

    F32 = mybir.dt.float32
    F16 = mybir.dt.float16
    AF = mybir.ActivationFunctionType
    ALU = mybir.AluOpType

    nc = bacc.Bacc(None, target_bir_lowering=False)

    xt_d = nc.dram_tensor("xt", [128, SPC * HC * L], F16, kind="ExternalInput")
    # weights packed k-strip-major so each weight DMA is 128 large
    # contiguous descriptors
    wt0_d = nc.dram_tensor("wt0", [128, 1, HC, 128], F16, kind="ExternalInput")
    wt1_d = nc.dram_tensor("wt1", [128, 1, HC, 128], F16, kind="ExternalInput")
    wt2_d = nc.dram_tensor("wt2", [128, 1, HC, 128], F16, kind="ExternalInput")
    wtr_d = nc.dram_tensor("wtr", [128, 3, HC, 128], F16, kind="ExternalInput")
    bfc_d = nc.dram_tensor("bfc", [128, HC], F32, kind="ExternalInput")
    wnu_d = nc.dram_tensor("wnu", [128, HC], F16, kind="ExternalInput")
    out_d = nc.dram_tensor("out", [SPC, H], F32, kind="ExternalOutput")

    with tile.TileContext(nc) as tc:
        with tc.tile_pool(name="xp", bufs=1) as xp, \
             tc.tile_pool(name="wp", bufs=1) as wp, \
             tc.tile_pool(name="cst", bufs=1) as cst, \
             tc.tile_pool(name="ups", bufs=2) as upsp, \
             tc.tile_pool(name="sm", bufs=2) as smp, \
             tc.tile_pool(name="outp", bufs=2) as outp, \
             tc.tile_pool(name="mmps", bufs=4, space="PSUM") as mmps, \
             tc.tile_pool(name="nups", bufs=2, space="PSUM") as nups, \
             tc.tile_pool(name="tps", bufs=2, space="PSUM") as tps:

            # ---- PE warmup: junk matmuls with no DMA dependency ramp the
            # p-state and the HAM activity window while the first tiles
            # stream in.
            wu_sb = cst.tile([128, L], F16)
            nc.vector.memset(wu_sb[:], 1.0)
            wu_ps = tps.tile([128, L], F32, tag="tp", name="wu_ps")
            for i in range(WARMUP_MM):
                nc.tensor.matmul(wu_ps[:], wu_sb[:, 0:128], wu_sb[:],
                                 start=(i == 0), stop=(i == WARMUP_MM - 1))

            # ---- DMA: tiny constants on the gpsimd direct queue; weights
            # and X^T through the sync HW queue, interleaved so the first
            # sample's matmuls start as early as possible.
            bfc_sb = cst.tile([128, HC], F32)
            wnu_sb = cst.tile([128, HC], F16)
            wt_sb = wp.tile([128, HC, HC, 128], F16)   # [p, kstrip, h, m]
            xt_sb = xp.tile([128, SPC * HC * L], F16)
            ident = cst.tile([128, 128], F32)

            nc.gpsimd.dma_start(bfc_sb[:], bfc_d[:])
            nc.gpsimd.dma_start(wnu_sb[:], wnu_d[:])

            def xt_sl(s, h):
                return xt_sb[:, (s * HC + h) * L:(s * HC + h + 1) * L]

            nc.sync.dma_start(wt_sb[:, 0:1, :, :], wt0_d[:])
            nc.sync.dma_start(xt_sb[:, 0:3 * L], xt_d[:, 0:3 * L])
            nc.sync.dma_start(xt_sb[:, 3 * L:HC * L], xt_d[:, 3 * L:HC * L])
            nc.sync.dma_start(wt_sb[:, 1:2, :, :], wt1_d[:])
            nc.sync.dma_start(wt_sb[:, 2:3, :, :], wt2_d[:])
            nc.sync.dma_start(wt_sb[:, 3:6, :, :], wtr_d[:])
            for s in range(1, SPC):
                nc.sync.dma_start(xt_sb[:, s * HC * L:(s + 1) * HC * L],
                                  xt_d[:, s * HC * L:(s + 1) * HC * L])
            make_identity(nc, ident[:])

            # ---- per-sample state carried to later emission points
            ups_t = [None] * SPC
            # all samples' pooled vectors gather into one [128, 48] tile,
            # drained by a single transpose + copy + DMA at the end
            pucat = outp.tile([128, SPC * HC], F32, tag="pucat")

            def emit_group(s, k, ups, c0, c1):
                """one k-chunk matmul group + tanh for tokens [c0, c1)."""
                ps = mmps.tile([128, L], F32, tag="mm")
                w = c1 - c0
                for h in range(HC):
                    nc.tensor.matmul(
                        ps[:, 0:w],
                        wt_sb[:, k, h, :],
                        xt_sl(s, h)[:, c0:c1],
                        start=(h == 0),
                        stop=(h == HC - 1),
                    )
                nc.scalar.activation(
                    ups[:, k, c0:c1], ps[:, 0:w], AF.Tanh,
                    bias=bfc_sb[:, k:k + 1],
                )

            def emit_epilogue(s):
                """nu + softmax + pooling for sample s, emitted inside a
                later sample's matmul stream where all inputs are done."""
                ups = ups_t[s]
                nu = nups.tile([1, L], F32, tag="nu", name="nu_p")
                for k in range(HC):
                    nc.tensor.matmul(
                        nu[:], wnu_sb[:, k:k + 1], ups[:, k, :],
                        start=(k == 0), stop=(k == HC - 1),
                    )
                # nu is small enough that exp() needs no max subtraction
                ex = smp.tile([1, L], F16, tag="ex")
                z = smp.tile([1, 1], F32, tag="z")
                rz = smp.tile([1, 1], F32, tag="rz")
                nc.scalar.activation(ex[:], nu[:], AF.Exp, accum_out=z[:])
                nc.vector.reciprocal(rz[:], z[:])
                ab = smp.tile([128, L], F16, tag="ab")
                nc.gpsimd.partition_broadcast(ab[:], ex[:])
                rzb = smp.tile([128, 1], F32, tag="rzb")
                nc.gpsimd.partition_broadcast(rzb[:], rz[:])
                # weighted-sum pooling on the VectorEngine; the 1/Z
                # normalization rides the STT per-partition scalar
                for h in range(HC):
                    trash = smp.tile([128, L], F16, tag="trash")
                    nc.vector.scalar_tensor_tensor(
                        trash[:], xt_sl(s, h), rzb[:, 0:1], ab[:],
                        ALU.mult, ALU.mult,
                        accum_out=pucat[:, s * HC + h:s * HC + h + 1],
                    )

            # ---- samples 0..6: full-width pipeline
            for s in range(SPC - 1):
                ups = upsp.tile([128, HC, L], F16, tag="ups")
                ups_t[s] = ups
                for ji in range(HC):
                    emit_group(s, ji, ups, 0, L)
                    if s > 0 and ji == 1:
                        emit_epilogue(s - 1)

            # ---- last sample: two 256-token halves so the softmax/pool
            # chain of half 0 overlaps half 1's matmuls, and only a short
            # chain trails the final matmul
            s = SPC - 1
            ups = upsp.tile([128, HC, L], F16, tag="ups", name="ups_last")
            ups_t[s] = ups
            korder0 = list(range(HC))
            korder1 = [5, 0, 1, 2, 3, 4]
            nu_a = nups.tile([1, L], F32, tag="nu", name="nu_a")
            nu_b = None
            ex = smp.tile([1, L], F16, tag="ex", name="ex_l")
            ab = smp.tile([128, L], F16, tag="ab", name="ab_l")
            z0 = smp.tile([1, 1], F32, tag="z", name="z0")
            z1 = smp.tile([1, 1], F32, tag="z", name="z1")
            pu2 = outp.tile([128, 2 * HC], F32, tag="pu2")

            # half 0
            for ji, k in enumerate(korder0):
                emit_group(s, k, ups, 0, HL)
                if ji == 1:
                    emit_epilogue(s - 1)
                if ji >= 2:
                    kk = korder0[ji - 2]
                    nc.tensor.matmul(nu_a[:, 0:HL], wnu_sb[:, kk:kk + 1],
                                     ups[:, kk, 0:HL],
                                     start=(ji == 2), stop=False)
            # half 1 (k=5 first so tanh(k=4) barely gates the nu tail)
            for ji, k in enumerate(korder1):
                emit_group(s, k, ups, HL, L)
                if ji == 0:
                    kk = korder0[4]
                    nc.tensor.matmul(nu_a[:, 0:HL], wnu_sb[:, kk:kk + 1],
                                     ups[:, kk, 0:HL], start=False, stop=False)
                if ji == 1:
                    kk = korder0[5]
                    nc.tensor.matmul(nu_a[:, 0:HL], wnu_sb[:, kk:kk + 1],
                                     ups[:, kk, 0:HL], start=False, stop=True)
                    # half-0 epilogue: unnormalized pooling into pu2[:, 0:6]
                    nc.scalar.activation(ex[:, 0:HL], nu_a[:, 0:HL], AF.Exp,
                                         accum_out=z0[:])
                    nc.gpsimd.partition_broadcast(ab[:, 0:HL], ex[:, 0:HL])
                    for h in range(HC):
                        trash = smp.tile([128, L], F16, tag="trash",
                                         name=f"tr0{h}")
                        nc.vector.scalar_tensor_tensor(
                            trash[:, 0:HL], xt_sl(s, h)[:, 0:HL], 1.0,
                            ab[:, 0:HL], ALU.mult, ALU.mult,
                            accum_out=pu2[:, h:h + 1],
                        )
                if ji >= 2:
                    kk = korder1[ji - 2]
                    if nu_b is None:
                        nu_b = nups.tile([1, L], F32, tag="nu", name="nu_b")
                    nc.tensor.matmul(nu_b[:, 0:HL], wnu_sb[:, kk:kk + 1],
                                     ups[:, kk, HL:L],
                                     start=(ji == 2), stop=False)
            for i, kk in enumerate([korder1[4], korder1[5]]):
                nc.tensor.matmul(nu_b[:, 0:HL], wnu_sb[:, kk:kk + 1],
                                 ups[:, kk, HL:L], start=False, stop=(i == 1))
            # half-1 epilogue: normalized pooling into pu2[:, 6:12]
            zs = smp.tile([1, 1], F32, tag="zs")
            rz = smp.tile([1, 1], F32, tag="rz", name="rz_l")
            rzb = smp.tile([128, 1], F32, tag="rzb", name="rzb_l")
            nc.scalar.activation(ex[:, HL:L], nu_b[:, 0:HL], AF.Exp,
                                 accum_out=z1[:])
            nc.vector.tensor_tensor(zs[:], z0[:], z1[:], ALU.add)
            nc.vector.reciprocal(rz[:], zs[:])
            nc.gpsimd.partition_broadcast(ab[:, HL:L], ex[:, HL:L])
            nc.gpsimd.partition_broadcast(rzb[:], rz[:])
            for h in range(HC):
                trash = smp.tile([128, L], F16, tag="trash", name=f"tr1{h}")
                nc.vector.scalar_tensor_tensor(
                    trash[:, 0:HL], xt_sl(s, h)[:, HL:L], rzb[:, 0:1],
                    ab[:, HL:L], ALU.mult, ALU.mult,
                    accum_out=pu2[:, HC + h:HC + h + 1],
                )
            # combine: pooled = pu_half0 * rz + pu_half1 (already * rz)
            nc.vector.scalar_tensor_tensor(
                pucat[:, s * HC:(s + 1) * HC], pu2[:, 0:HC], rzb[:, 0:1],
                pu2[:, HC:2 * HC], ALU.mult, ALU.add,
            )
            # single gathered drain: [128, 48] -> [48, 128] -> DRAM
            tp_all = tps.tile([SPC * HC, 128], F32, tag="tp", name="tp_all")
            nc.tensor.transpose(tp_all[:], pucat[:], ident[:])
            orow = outp.tile([SPC * HC, 128], F32, tag="orow")
            nc.scalar.copy(orow[:], tp_all[:])
            nc.sync.dma_start(
                out_d[:, :].rearrange("s (c p) -> (s c) p", p=128),
                orow[:],
            )

    nc.finalize()
    return nc


def kernel(hidden_states, W_fc, b_fc, W_nu, _trace=False, _trace_kwargs=None):
    from concourse.bass_utils import run_bass_kernel_spmd

    hs = np.ascontiguousarray(hidden_states, dtype=np.float32)
    W_fc = np.asarray(W_fc, np.float32)
    b_fc = np.asarray(b_fc, np.float32)
    W_nu = np.asarray(W_nu, np.float32)

    # W^T in [128, kstrip, h, m] layout:
    # [p, ks, h, m] = W_fc[ks*128+m, h*128+p]
    wth = np.ascontiguousarray(
        W_fc.T.reshape(HC, 128, HC, 128).transpose(1, 2, 0, 3)
        .astype(np.float16))
    wt0_host = np.ascontiguousarray(wth[:, 0:1])
    wt1_host = np.ascontiguousarray(wth[:, 1:2])
    wt2_host = np.ascontiguousarray(wth[:, 2:3])
    wtr_host = np.ascontiguousarray(wth[:, 3:6])
    bfc_host = np.ascontiguousarray(b_fc.reshape(HC, 128).T, np.float32)
    wnu_host = np.ascontiguousarray(W_nu.reshape(HC, 128).T.astype(np.float16))

    in_maps = []
    for c in range(NCORES):
        # X^T in sample-major [128, (s c t)] layout so each per-sample DMA
        # is 128 contiguous 6KB descriptors:
        # [p, s, c, t] = X[s*512+t, c*128+p]
        xt = np.ascontiguousarray(
            hs[c * SPC:(c + 1) * SPC].reshape(TOK, H).T
            .reshape(HC, 128, SPC, L).transpose(1, 2, 0, 3)
            .reshape(128, SPC * HC * L).astype(np.float16))
        in_maps.append({"xt": xt, "wt0": wt0_host, "wt1": wt1_host,
                        "wt2": wt2_host, "wtr": wtr_host,
                        "bfc": bfc_host, "wnu": wnu_host})

    if "nc" not in _compiled:
        _compiled["nc"] = _build()
    res = run_bass_kernel_spmd(
        _compiled["nc"], in_maps, list(range(NCORES)),
        trace=_trace, **(_trace_kwargs or {}),
    )
    kernel.last_results = res
    out = np.concatenate([np.asarray(r["out"], np.float32) for r in res.results])
    return out



# revision 5
# speedup vs baseline: 1.1593x; 1.1593x over previous
"""AttentionProtoNet pooling kernel for 8x TRN2 NeuronCores.

reference (per sample of B=64, L=512, H=768):
    upsilon = tanh(hs @ W_fc.T + b_fc)        [L, H]
    nu      = upsilon @ W_nu                  [L]
    alphas  = softmax(nu)                     [L]
    pooled  = alphas @ hs                     [H]

Strategy: data-parallel over B (8 samples per core).

Mixed-precision main matmul: the FC output dim is permuted on the host so
|W_nu| is ascending; the 5 output chunks carrying ~42% of the W_nu^2 mass
run as fp8e4 DoubleRow matmuls (2 contraction chunks per MM, ~732ns per
output chunk) while the heaviest chunk (58% of the mass) stays fp16
(1296ns). nu-noise from fp8 quantization scales with the W_nu^2 mass run
in fp8, keeping the end-to-end rel err ~1.2e-2 (<2e-2 gate) while cutting
the dominant PE cost by ~36%.

nu (samples 0..6) runs as column-tiled M=1 matmuls: chunk pairs go to PE
column groups 0/32 concurrently (span ~3 MMs instead of 6), and the two
partial rows are summed on the DVE.

DMA: two HWDGE queues in parallel (scalar queue: weights + fp16 X; sync
queue: fp8 X), ordered so the first sample's fp8 matmuls start as early
as possible. Pooling reads the fp16 X copy (fp8 X would break precision).
The per-sample nu/softmax/pooling epilogue is emitted inside the NEXT
sample's matmul stream; the last sample runs in two 256-token halves so
its softmax/pooling chain overlaps its own matmuls.
"""

import sys

sys.path.insert(0, "/opt/trn_rl_repo")

import numpy as np
import ml_dtypes

B, L, H = 64, 512, 768
NCORES = 8
SPC = B // NCORES            # samples per core
TOK = SPC * L                # tokens per core
HC = H // 128                # 128-partition chunks of H
NK8 = 5                      # fp8 output chunks (sorted by |wnu| asc)
NP = HC // 2                 # contraction pairs for DoubleRow
HL = L // 2                  # token half for the last sample
WARMUP_MM = 14               # junk matmuls: p-state + HAM ramp during DMA

_compiled = {}


def _build():
    import concourse.bass as bass
    import concourse.bacc as bacc
    import concourse.tile as tile
    from concourse import mybir
    from concourse.masks import make_identity

    F32 = mybir.dt.float32
    F16 = mybir.dt.float16
    F8 = mybir.dt.float8e4
    AF = mybir.ActivationFunctionType
    ALU = mybir.AluOpType
    DR = mybir.MatmulPerfMode.DoubleRow

    nc = bacc.Bacc(None, target_bir_lowering=False)

    xt8_d = nc.dram_tensor("xt8", [128, SPC * HC * L], F8, kind="ExternalInput")
    xt16_d = nc.dram_tensor("xt16", [128, SPC * HC * L], F16,
                            kind="ExternalInput")
    # fp8 weights for kouts 0..4: [p, k*3+j, i, m] = Wp[k*128+m, (2j+i)*128+p]
    wt8_d = nc.dram_tensor("wt8", [128, NK8 * NP, 2, 128], F8,
                           kind="ExternalInput")
    # fp16 weights for kout 5: [p, h, m] = Wp[5*128+m, h*128+p]
    wt16_d = nc.dram_tensor("wt16", [128, HC, 128], F16, kind="ExternalInput")
    bfc_d = nc.dram_tensor("bfc", [128, HC], F32, kind="ExternalInput")
    wnu_d = nc.dram_tensor("wnu", [128, HC], F16, kind="ExternalInput")
    out_d = nc.dram_tensor("out", [SPC, H], F32, kind="ExternalOutput")

    with tile.TileContext(nc) as tc:
        with tc.tile_pool(name="xp", bufs=1) as xp, \
             tc.tile_pool(name="wp", bufs=1) as wp, \
             tc.tile_pool(name="cst", bufs=1) as cst, \
             tc.tile_pool(name="ups", bufs=2) as upsp, \
             tc.tile_pool(name="sm", bufs=2) as smp, \
             tc.tile_pool(name="outp", bufs=2) as outp, \
             tc.tile_pool(name="mmps", bufs=4, space="PSUM") as mmps, \
             tc.tile_pool(name="nups", bufs=2, space="PSUM") as nups, \
             tc.tile_pool(name="tps", bufs=2, space="PSUM") as tps:

            # ---- PE warmup: junk matmuls with no DMA dependency ramp the
            # p-state and the HAM activity window while the first tiles
            # stream in.
            wu_sb = cst.tile([128, L], F16)
            nc.vector.memset(wu_sb[:], 1.0)
            wu_ps = tps.tile([128, L], F32, tag="tp", name="wu_ps")
            for i in range(WARMUP_MM):
                nc.tensor.matmul(wu_ps[:], wu_sb[:, 0:128], wu_sb[:],
                                 start=(i == 0), stop=(i == WARMUP_MM - 1))

            # ---- SBUF tiles
            bfc_sb = cst.tile([128, HC], F32)
            wnu_sb = cst.tile([128, HC], F16)
            wt8_sb = wp.tile([128, NK8 * NP, 2, 128], F8)
            wt16_sb = wp.tile([128, HC, 128], F16)
            xt8_sb = xp.tile([128, SPC * HC, L], F8)
            xt16_sb = xp.tile([128, SPC * HC, L], F16)
            ident = cst.tile([128, 128], F32)

            # ---- DMA: two HWDGE queues in parallel.
            # scalar queue: weights only (5 short gens; the first fp8 MM is
            # gated on wt8[k=0], the first tanh on bfc)
            nc.scalar.dma_start(wt8_sb[:, 0:NP], wt8_d[:, 0:NP])
            nc.scalar.dma_start(bfc_sb[:], bfc_d[:])
            nc.scalar.dma_start(wnu_sb[:], wnu_d[:])
            nc.scalar.dma_start(wt8_sb[:, NP:NK8 * NP], wt8_d[:, NP:NK8 * NP])
            nc.scalar.dma_start(wt16_sb[:], wt16_d[:])

            # sync queue: X, fp8 before fp16 per sample (sample 0's fp8
            # split so the first MM starts as early as possible)
            def dma_xt(sb, d, s, c0, c1):
                nc.sync.dma_start(
                    sb[:, s * HC + c0:s * HC + c1, :],
                    d[:, (s * HC + c0) * L:(s * HC + c1) * L].rearrange(
                        "p (c t) -> p c t", t=L),
                )
            dma_xt(xt8_sb, xt8_d, 0, 0, 2)
            dma_xt(xt8_sb, xt8_d, 0, 2, HC)
            dma_xt(xt16_sb, xt16_d, 0, 0, HC)
            for s in range(1, SPC):
                dma_xt(xt8_sb, xt8_d, s, 0, HC)
                dma_xt(xt16_sb, xt16_d, s, 0, HC)
            make_identity(nc, ident[:])

            # ---- per-sample state carried to later emission points
            ups_t = [None] * SPC
            # all samples' pooled vectors gather into one [128, 48] tile,
            # drained by a single transpose + copy + DMA at the end
            pucat = outp.tile([128, SPC * HC], F32, tag="pucat")

            def emit_group8(s, k, ups, c0, c1):
                """fp8 DoubleRow matmul group + tanh, kout k, tokens [c0,c1)."""
                ps = mmps.tile([128, L], F32, tag="mm")
                w = c1 - c0
                for j in range(NP):
                    nc.tensor.matmul(
                        ps[:, 0:w],
                        wt8_sb[:, k * NP + j],
                        xt8_sb[:, s * HC + 2 * j:s * HC + 2 * j + 2, c0:c1],
                        start=(j == 0),
                        stop=(j == NP - 1),
                        perf_mode=DR,
                    )
                nc.scalar.activation(
                    ups[:, k, c0:c1], ps[:, 0:w], AF.Tanh,
                    bias=bfc_sb[:, k:k + 1],
                )

            def emit_group16(s, k, ups, c0, c1):
                """fp16 matmul group + tanh for kout k, tokens [c0, c1)."""
                ps = mmps.tile([128, L], F32, tag="mm")
                w = c1 - c0
                for h in range(HC):
                    nc.tensor.matmul(
                        ps[:, 0:w],
                        wt16_sb[:, h],
                        xt16_sb[:, s * HC + h, c0:c1],
                        start=(h == 0),
                        stop=(h == HC - 1),
                    )
                nc.scalar.activation(
                    ups[:, k, c0:c1], ps[:, 0:w], AF.Tanh,
                    bias=bfc_sb[:, k:k + 1],
                )

            def emit_epilogue(s):
                """nu + softmax + pooling for sample s, emitted inside a
                later sample's matmul stream where all inputs are done."""
                ups = ups_t[s]
                nu = nups.tile([1, L], F32, tag="nu", name=f"nu_{s}")
                for k in range(HC):
                    nc.tensor.matmul(
                        nu[:], wnu_sb[:, k:k + 1], ups[:, k, :],
                        start=(k == 0), stop=(k == HC - 1),
                    )
                # nu is small enough that exp() needs no max subtraction
                ex = smp.tile([1, L], F16, tag="ex")
                z = smp.tile([1, 1], F32, tag="z")
                rz = smp.tile([1, 1], F32, tag="rz")
                nc.scalar.activation(ex[:], nu[:], AF.Exp, accum_out=z[:])
                nc.vector.reciprocal(rz[:], z[:])
                ab = smp.tile([128, L], F16, tag="ab")
                nc.gpsimd.partition_broadcast(ab[:], ex[:])
                rzb = smp.tile([128, 1], F32, tag="rzb")
                nc.gpsimd.partition_broadcast(rzb[:], rz[:])
                # weighted-sum pooling on the VectorEngine; the 1/Z
                # normalization rides the STT per-partition scalar
                for h in range(HC):
                    trash = smp.tile([128, L], F16, tag="trash")
                    nc.vector.scalar_tensor_tensor(
                        trash[:], xt16_sb[:, s * HC + h, :], rzb[:, 0:1],
                        ab[:], ALU.mult, ALU.mult,
                        accum_out=pucat[:, s * HC + h:s * HC + h + 1],
                    )

            # ---- samples 0..6: full-width pipeline
            for s in range(SPC - 1):
                ups = upsp.tile([128, HC, L], F16, tag="ups")
                ups_t[s] = ups
                for k in range(NK8):
                    emit_group8(s, k, ups, 0, L)
                    if s > 0 and k == 1:
                        emit_epilogue(s - 1)
                emit_group16(s, HC - 1, ups, 0, L)

            # ---- last sample: two 256-token halves so the softmax/pool
            # chain of half 0 overlaps half 1's matmuls, and only a short
            # chain trails the final matmul
            s = SPC - 1
            ups = upsp.tile([128, HC, L], F16, tag="ups", name="ups_last")
            ups_t[s] = ups
            korder0 = list(range(HC))
            korder1 = [5, 0, 1, 2, 3, 4]

            def emit_group(s, k, ups, c0, c1):
                if k < NK8:
                    emit_group8(s, k, ups, c0, c1)
                else:
                    emit_group16(s, k, ups, c0, c1)

            nu_a = nups.tile([1, L], F32, tag="nu", name="nu_a")
            nu_b = None
            ex = smp.tile([1, L], F16, tag="ex", name="ex_l")
            ab = smp.tile([128, L], F16, tag="ab", name="ab_l")
            z0 = smp.tile([1, 1], F32, tag="z", name="z0")
            z1 = smp.tile([1, 1], F32, tag="z", name="z1")
            pu2 = outp.tile([128, 2 * HC], F32, tag="pu2")

            # half 0
            for ji, k in enumerate(korder0):
                emit_group(s, k, ups, 0, HL)
                if ji == 1:
                    emit_epilogue(s - 1)
                if ji >= 2:
                    kk = korder0[ji - 2]
                    nc.tensor.matmul(nu_a[:, 0:HL], wnu_sb[:, kk:kk + 1],
                                     ups[:, kk, 0:HL],
                                     start=(ji == 2), stop=False)
            # half 1 (k=5 first so tanh(k=4) barely gates the nu tail)
            for ji, k in enumerate(korder1):
                emit_group(s, k, ups, HL, L)
                if ji == 0:
                    kk = korder0[4]
                    nc.tensor.matmul(nu_a[:, 0:HL], wnu_sb[:, kk:kk + 1],
                                     ups[:, kk, 0:HL], start=False, stop=False)
                if ji == 1:
                    kk = korder0[5]
                    nc.tensor.matmul(nu_a[:, 0:HL], wnu_sb[:, kk:kk + 1],
                                     ups[:, kk, 0:HL], start=False, stop=True)
                    # half-0 epilogue: unnormalized pooling into pu2[:, 0:6]
                    nc.scalar.activation(ex[:, 0:HL], nu_a[:, 0:HL], AF.Exp,
                                         accum_out=z0[:])
                    nc.gpsimd.partition_broadcast(ab[:, 0:HL], ex[:, 0:HL])
                    for h in range(HC):
                        trash = smp.tile([128, L], F16, tag="trash",
                                         name=f"tr0{h}")
                        nc.vector.scalar_tensor_tensor(
                            trash[:, 0:HL], xt16_sb[:, s * HC + h, 0:HL], 1.0,
                            ab[:, 0:HL], ALU.mult, ALU.mult,
                            accum_out=pu2[:, h:h + 1],
                        )
                if ji >= 2:
                    kk = korder1[ji - 2]
                    if nu_b is None:
                        nu_b = nups.tile([1, L], F32, tag="nu", name="nu_b")
                    nc.tensor.matmul(nu_b[:, 0:HL], wnu_sb[:, kk:kk + 1],
                                     ups[:, kk, HL:L],
                                     start=(ji == 2), stop=False)
            for i, kk in enumerate([korder1[4], korder1[5]]):
                nc.tensor.matmul(nu_b[:, 0:HL], wnu_sb[:, kk:kk + 1],
                                 ups[:, kk, HL:L], start=False, stop=(i == 1))
            # half-1 epilogue: normalized pooling into pu2[:, 6:12]
            zs = smp.tile([1, 1], F32, tag="zs")
            rz = smp.tile([1, 1], F32, tag="rz", name="rz_l")
            rzb = smp.tile([128, 1], F32, tag="rzb", name="rzb_l")
            nc.scalar.activation(ex[:, HL:L], nu_b[:, 0:HL], AF.Exp,
                                 accum_out=z1[:])
            nc.vector.tensor_tensor(zs[:], z0[:], z1[:], ALU.add)
            nc.vector.reciprocal(rz[:], zs[:])
            nc.gpsimd.partition_broadcast(ab[:, HL:L], ex[:, HL:L])
            nc.gpsimd.partition_broadcast(rzb[:], rz[:])
            for h in range(HC):
                trash = smp.tile([128, L], F16, tag="trash", name=f"tr1{h}")
                nc.vector.scalar_tensor_tensor(
                    trash[:, 0:HL], xt16_sb[:, s * HC + h, HL:L], rzb[:, 0:1],
                    ab[:, HL:L], ALU.mult, ALU.mult,
                    accum_out=pu2[:, HC + h:HC + h + 1],
                )
            # combine: pooled = pu_half0 * rz + pu_half1 (already * rz)
            nc.vector.scalar_tensor_tensor(
                pucat[:, s * HC:(s + 1) * HC], pu2[:, 0:HC], rzb[:, 0:1],
                pu2[:, HC:2 * HC], ALU.mult, ALU.add,
            )
            # single gathered drain: [128, 48] -> [48, 128] -> DRAM
            tp_all = tps.tile([SPC * HC, 128], F32, tag="tp", name="tp_all")
            nc.tensor.transpose(tp_all[:], pucat[:], ident[:])
            orow = outp.tile([SPC * HC, 128], F32, tag="orow")
            nc.scalar.copy(orow[:], tp_all[:])
            nc.sync.dma_start(
                out_d[:, :].rearrange("s (c p) -> (s c) p", p=128),
                orow[:],
            )

    nc.finalize()
    return nc


def kernel(hidden_states, W_fc, b_fc, W_nu, _trace=False, _trace_kwargs=None):
    from concourse.bass_utils import run_bass_kernel_spmd

    E4 = ml_dtypes.float8_e4m3

    hs = np.ascontiguousarray(hidden_states, dtype=np.float32)
    W_fc = np.asarray(W_fc, np.float32)
    b_fc = np.asarray(b_fc, np.float32)
    W_nu = np.asarray(W_nu, np.float32)

    # permute the FC output dim so |W_nu| is ascending; fp8 noise then
    # lands on the low-|W_nu| output chunks
    perm = np.argsort(np.abs(W_nu), kind="stable")
    Wp = W_fc[perm]
    bp = b_fc[perm]
    wnup = W_nu[perm]

    # W^T chunk layouts: wth[p, kout, cin, m] = Wp[kout*128+m, cin*128+p]
    wth = Wp.T.reshape(HC, 128, HC, 128).transpose(1, 2, 0, 3)
    wt8_host = np.ascontiguousarray(
        wth[:, 0:NK8, :, :]                              # [p, k, cin, m]
        .reshape(128, NK8, NP, 2, 128)                   # pair cin chunks
        .reshape(128, NK8 * NP, 2, 128).astype(E4))
    wt16_host = np.ascontiguousarray(wth[:, HC - 1, :, :].astype(np.float16))
    bfc_host = np.ascontiguousarray(bp.reshape(HC, 128).T, np.float32)
    wnu_host = np.ascontiguousarray(wnup.reshape(HC, 128).T.astype(np.float16))

    in_maps = []
    for c in range(NCORES):
        # X^T in sample-major [128, (s c t)] layout so each per-sample DMA
        # is 128 contiguous descriptors: [p, s, c, t] = X[s*512+t, c*128+p]
        xt = np.ascontiguousarray(
            hs[c * SPC:(c + 1) * SPC].reshape(TOK, H).T
            .reshape(HC, 128, SPC, L).transpose(1, 2, 0, 3)
            .reshape(128, SPC * HC * L))
        in_maps.append({
            "xt8": xt.astype(E4),
            "xt16": xt.astype(np.float16),
            "wt8": wt8_host, "wt16": wt16_host,
            "bfc": bfc_host, "wnu": wnu_host,
        })

    if "nc" not in _compiled:
        _compiled["nc"] = _build()
    res = run_bass_kernel_spmd(
        _compiled["nc"], in_maps, list(range(NCORES)),
        trace=_trace, **(_trace_kwargs or {}),
    )
    kernel.last_results = res
    out = np.concatenate([np.asarray(r["out"], np.float32) for r in res.results])
    return out


# revision 11
# speedup vs baseline: 1.2577x; 1.0849x over previous
"""AttentionProtoNet pooling kernel for 8x TRN2 NeuronCores.

reference (per sample of B=64, L=512, H=768):
    upsilon = tanh(hs @ W_fc.T + b_fc)        [L, H]
    nu      = upsilon @ W_nu                  [L]
    alphas  = softmax(nu)                     [L]
    pooled  = alphas @ hs                     [H]

Strategy: data-parallel over B (8 samples per core).

Mixed-precision main matmul: the FC output dim is permuted on the host so
|W_nu| is ascending; the 5 output chunks carrying ~42% of the W_nu^2 mass
run as fp8e4 DoubleRow matmuls (2 contraction chunks per MM, ~732ns per
output chunk) while the heaviest chunk (58% of the mass) stays fp16
(1296ns). nu-noise from fp8 quantization scales with the W_nu^2 mass run
in fp8, keeping the end-to-end rel err ~1.2e-2 (<2e-2 gate) while cutting
the dominant PE cost by ~36%.

nu (samples 0..6) runs as column-tiled M=1 matmuls: chunk pairs go to PE
column groups 0/32 concurrently (span ~3 MMs instead of 6), and the two
partial rows are summed on the DVE.

DMA: two HWDGE queues in parallel (scalar queue: weights + fp16 X; sync
queue: fp8 X), ordered so the first sample's fp8 matmuls start as early
as possible. Pooling reads the fp16 X copy (fp8 X would break precision).
The per-sample nu/softmax/pooling epilogue is emitted inside the NEXT
sample's matmul stream; the last sample runs in two 256-token halves so
its softmax/pooling chain overlaps its own matmuls.
"""

import sys

sys.path.insert(0, "/opt/trn_rl_repo")

import numpy as np
import ml_dtypes

B, L, H = 64, 512, 768
NCORES = 8
SPC = B // NCORES            # samples per core
TOK = SPC * L                # tokens per core
HC = H // 128                # 128-partition chunks of H
NK8 = 5                      # fp8 output chunks (sorted by |wnu| asc)
NP = HC // 2                 # contraction pairs for DoubleRow
HL = L // 2                  # token half for the last sample
WARMUP_MM = 14               # junk matmuls: p-state + HAM ramp during DMA

_compiled = {}


def _build():
    import concourse.bass as bass
    import concourse.bacc as bacc
    import concourse.tile as tile
    from concourse import mybir
    from concourse.masks import make_identity

    F32 = mybir.dt.float32
    F16 = mybir.dt.float16
    F8 = mybir.dt.float8e4
    AF = mybir.ActivationFunctionType
    ALU = mybir.AluOpType
    DR = mybir.MatmulPerfMode.DoubleRow

    nc = bacc.Bacc(None, target_bir_lowering=False)

    xt8_d = nc.dram_tensor("xt8", [128, SPC * HC * L], F8, kind="ExternalInput")
    xt16_d = nc.dram_tensor("xt16", [128, SPC * HC * L], F16,
                            kind="ExternalInput")
    # fp8 weights for kouts 0..4: [p, k*3+j, i, m] = Wp[k*128+m, (2j+i)*128+p]
    wt8_d = nc.dram_tensor("wt8", [128, NK8 * NP, 2, 128], F8,
                           kind="ExternalInput")
    # fp16 weights for kout 5: [p, h, m] = Wp[5*128+m, h*128+p]
    wt16_d = nc.dram_tensor("wt16", [128, HC, 128], F16, kind="ExternalInput")
    bfc_d = nc.dram_tensor("bfc", [128, HC], F32, kind="ExternalInput")
    wnu_d = nc.dram_tensor("wnu", [128, HC], F16, kind="ExternalInput")
    out_d = nc.dram_tensor("out", [SPC, H], F32, kind="ExternalOutput")

    with tile.TileContext(nc) as tc:
        with tc.tile_pool(name="xp", bufs=1) as xp, \
             tc.tile_pool(name="wp", bufs=1) as wp, \
             tc.tile_pool(name="cst", bufs=1) as cst, \
             tc.tile_pool(name="ups", bufs=2) as upsp, \
             tc.tile_pool(name="sm", bufs=2) as smp, \
             tc.tile_pool(name="outp", bufs=2) as outp, \
             tc.tile_pool(name="mmps", bufs=3, space="PSUM") as mmps, \
             tc.tile_pool(name="nups", bufs=2, space="PSUM") as nups, \
             tc.tile_pool(name="abp", bufs=1, space="PSUM") as abp, \
             tc.tile_pool(name="tps", bufs=2, space="PSUM") as tps:

            # ---- PE warmup: junk matmuls with no DMA dependency ramp the
            # p-state and the HAM activity window while the first tiles
            # stream in.
            wu_sb = cst.tile([128, L], F16)
            nc.vector.memset(wu_sb[:], 1.0)
            wu_ps = tps.tile([128, L], F32, tag="tp", name="wu_ps")
            for i in range(WARMUP_MM):
                nc.tensor.matmul(wu_ps[:], wu_sb[:, 0:128], wu_sb[:],
                                 start=(i == 0), stop=(i == WARMUP_MM - 1))

            # ---- SBUF tiles
            bfc_sb = cst.tile([128, HC], F32)
            wnu_sb = cst.tile([128, HC], F16)
            wt8_sb = wp.tile([128, NK8 * NP, 2, 128], F8)
            wt16_sb = wp.tile([128, HC, 128], F16)
            xt8_sb = xp.tile([128, SPC * HC, L], F8)
            xt16_sb = xp.tile([128, SPC * HC, L], F16)
            ident = cst.tile([128, 128], F32)

            # ---- DMA: ONE HWDGE queue, weights strictly first.  A second
            # queue round-robins packets against this one at the SDMA level
            # and starves the small weight transfers behind the X flood
            # (measured: first fp8 MM waited until 18us that way).
            nc.sync.dma_start(wt8_sb[:, 0:NP], wt8_d[:, 0:NP])
            nc.sync.dma_start(bfc_sb[:], bfc_d[:])
            nc.sync.dma_start(wnu_sb[:], wnu_d[:])
            nc.sync.dma_start(wt16_sb[:], wt16_d[:])
            nc.sync.dma_start(wt8_sb[:, NP:NK8 * NP], wt8_d[:, NP:NK8 * NP])

            # then X: fp8 before fp16 per sample (sample 0's fp8 split so
            # the first MM starts as early as possible)
            def dma_xt(sb, d, s, c0, c1):
                nc.sync.dma_start(
                    sb[:, s * HC + c0:s * HC + c1, :],
                    d[:, (s * HC + c0) * L:(s * HC + c1) * L].rearrange(
                        "p (c t) -> p c t", t=L),
                )
            dma_xt(xt8_sb, xt8_d, 0, 0, 2)
            dma_xt(xt8_sb, xt8_d, 0, 2, HC)
            dma_xt(xt16_sb, xt16_d, 0, 0, HC)
            for s in range(1, SPC):
                dma_xt(xt8_sb, xt8_d, s, 0, HC)
                dma_xt(xt16_sb, xt16_d, s, 0, HC)
            make_identity(nc, ident[:])

            # ---- per-sample state carried to later emission points
            ups_t = [None] * SPC
            # all samples' pooled vectors gather into one [128, 48] tile,
            # drained by a single transpose + copy + DMA at the end
            pucat = outp.tile([128, SPC * HC], F32, tag="pucat")

            def emit_group8(s, k, ups, c0, c1):
                """fp8 DoubleRow matmul group + tanh, kout k, tokens [c0,c1)."""
                ps = mmps.tile([128, L], F32, tag="mm")
                w = c1 - c0
                for j in range(NP):
                    nc.tensor.matmul(
                        ps[:, 0:w],
                        wt8_sb[:, k * NP + j],
                        xt8_sb[:, s * HC + 2 * j:s * HC + 2 * j + 2, c0:c1],
                        start=(j == 0),
                        stop=(j == NP - 1),
                        perf_mode=DR,
                    )
                nc.scalar.activation(
                    ups[:, k, c0:c1], ps[:, 0:w], AF.Tanh,
                    bias=bfc_sb[:, k:k + 1],
                )

            def emit_group16(s, k, ups, c0, c1):
                """fp16 matmul group + tanh for kout k, tokens [c0, c1)."""
                ps = mmps.tile([128, L], F32, tag="mm")
                w = c1 - c0
                for h in range(HC):
                    nc.tensor.matmul(
                        ps[:, 0:w],
                        wt16_sb[:, h],
                        xt16_sb[:, s * HC + h, c0:c1],
                        start=(h == 0),
                        stop=(h == HC - 1),
                    )
                nc.scalar.activation(
                    ups[:, k, c0:c1], ps[:, 0:w], AF.Tanh,
                    bias=bfc_sb[:, k:k + 1],
                )

            def emit_epilogue(s):
                """nu + softmax + pooling for sample s, emitted inside a
                later sample's matmul stream where all inputs are done."""
                ups = ups_t[s]
                nu = nups.tile([1, L], F32, tag="nu", name=f"nu_{s}")
                for k in range(HC):
                    nc.tensor.matmul(
                        nu[:], wnu_sb[:, k:k + 1], ups[:, k, :],
                        start=(k == 0), stop=(k == HC - 1),
                    )
                # nu is small enough that exp() needs no max subtraction
                ex = smp.tile([1, L], F16, tag="ex")
                z = smp.tile([1, 1], F32, tag="z")
                rz = smp.tile([1, 1], F32, tag="rz")
                nc.scalar.activation(ex[:], nu[:], AF.Exp, accum_out=z[:])
                nc.vector.reciprocal(rz[:], z[:])
                ab = smp.tile([128, L], F16, tag="ab")
                nc.gpsimd.partition_broadcast(ab[:], ex[:])
                rzb = smp.tile([128, 1], F32, tag="rzb")
                nc.gpsimd.partition_broadcast(rzb[:], rz[:])
                # weighted-sum pooling on the VectorEngine; the 1/Z
                # normalization rides the STT per-partition scalar
                for h in range(HC):
                    trash = smp.tile([128, L], F16, tag="trash")
                    nc.vector.scalar_tensor_tensor(
                        trash[:], xt16_sb[:, s * HC + h, :], rzb[:, 0:1],
                        ab[:], ALU.mult, ALU.mult,
                        accum_out=pucat[:, s * HC + h:s * HC + h + 1],
                    )

            # ---- samples 0..6: full-width pipeline
            for s in range(SPC - 1):
                ups = upsp.tile([128, HC, L], F16, tag="ups")
                ups_t[s] = ups
                for k in range(NK8):
                    emit_group8(s, k, ups, 0, L)
                    if s > 0 and k == 1:
                        emit_epilogue(s - 1)
                emit_group16(s, HC - 1, ups, 0, L)

            # ---- last sample: two 256-token halves so the softmax/pool
            # chain of half 0 overlaps half 1's matmuls, and only a short
            # chain trails the final matmul
            s = SPC - 1
            ups = upsp.tile([128, HC, L], F16, tag="ups", name="ups_last")
            ups_t[s] = ups
            korder0 = list(range(HC))
            korder1 = [5, 0, 1, 2, 3, 4]

            def emit_group(s, k, ups, c0, c1):
                if k < NK8:
                    emit_group8(s, k, ups, c0, c1)
                else:
                    emit_group16(s, k, ups, c0, c1)

            nu_a = nups.tile([1, L], F32, tag="nu", name="nu_a")
            nu_b = None
            ex = smp.tile([1, L], F16, tag="ex", name="ex_l")
            ab_ps = abp.tile([128, L], F32, tag="abps", name="ab_ps")
            z0 = smp.tile([1, 1], F32, tag="z", name="z0")
            z1 = smp.tile([1, 1], F32, tag="z", name="z1")
            pu2 = outp.tile([128, 2 * HC], F32, tag="pu2")

            # half 0
            for ji, k in enumerate(korder0):
                emit_group(s, k, ups, 0, HL)
                if ji == 1:
                    emit_epilogue(s - 1)
                if ji >= 2:
                    kk = korder0[ji - 2]
                    nc.tensor.matmul(nu_a[:, 0:HL], wnu_sb[:, kk:kk + 1],
                                     ups[:, kk, 0:HL],
                                     start=(ji == 2), stop=False)
            # half 1 (k=5 first so tanh(k=4) barely gates the nu tail)
            for ji, k in enumerate(korder1):
                emit_group(s, k, ups, HL, L)
                if ji == 0:
                    kk = korder0[4]
                    nc.tensor.matmul(nu_a[:, 0:HL], wnu_sb[:, kk:kk + 1],
                                     ups[:, kk, 0:HL], start=False, stop=False)
                if ji == 1:
                    kk = korder0[5]
                    nc.tensor.matmul(nu_a[:, 0:HL], wnu_sb[:, kk:kk + 1],
                                     ups[:, kk, 0:HL], start=False, stop=True)
                    # half-0 epilogue: exp broadcast via a K=1 ones matmul
                    # into PSUM (no gpsimd latency), unnormalized pooling
                    # into pu2[:, 0:6]; 3 chunks on DVE + 3 on gpsimd
                    nc.scalar.activation(ex[:, 0:HL], nu_a[:, 0:HL], AF.Exp,
                                         accum_out=z0[:])
                    nc.tensor.matmul(ab_ps[:, 0:HL], wu_sb[0:1, 0:128],
                                     ex[0:1, 0:HL], start=True, stop=True,
                                     skip_group_check=True)
                    for h in range(HC):
                        trash = smp.tile([128, L], F16, tag="trash",
                                         name=f"tr0{h}")
                        nc.vector.scalar_tensor_tensor(
                            trash[:, 0:HL], xt16_sb[:, s * HC + h, 0:HL], 1.0,
                            ab_ps[:, 0:HL], ALU.mult, ALU.mult,
                            accum_out=pu2[:, h:h + 1],
                        )
                if ji >= 2:
                    kk = korder1[ji - 2]
                    if nu_b is None:
                        nu_b = nups.tile([1, L], F32, tag="nu", name="nu_b")
                    nc.tensor.matmul(nu_b[:, 0:HL], wnu_sb[:, kk:kk + 1],
                                     ups[:, kk, HL:L],
                                     start=(ji == 2), stop=False)
            for i, kk in enumerate([korder1[4], korder1[5]]):
                nc.tensor.matmul(nu_b[:, 0:HL], wnu_sb[:, kk:kk + 1],
                                 ups[:, kk, HL:L], start=False, stop=(i == 1))
            # samples 0..6 drain early, during the last sample's tail: the
            # PE transpose sits here in queue order so its deps (sample 6's
            # pooling) are long satisfied and it never blocks the queue
            tp0 = tps.tile([(SPC - 1) * HC, 128], F32, tag="tp", name="tp0")
            nc.tensor.transpose(tp0[:], pucat[:, 0:(SPC - 1) * HC], ident[:])
            orow0 = outp.tile([(SPC - 1) * HC, 128], F32, tag="orow0")
            nc.vector.tensor_copy(orow0[:], tp0[:])
            out_r = out_d[:, :].rearrange("s (c p) -> (s c) p", p=128)
            nc.sync.dma_start(out_r[0:(SPC - 1) * HC, :], orow0[:])

            # half-1 epilogue: unnormalized pooling into pu2[:, 6:12];
            # normalization happens once on the tiny [128, 6] combine
            zs = smp.tile([1, 1], F32, tag="zs")
            rz = smp.tile([1, 1], F32, tag="rz", name="rz_l")
            rzb = smp.tile([128, 1], F32, tag="rzb", name="rzb_l")
            nc.scalar.activation(ex[:, HL:L], nu_b[:, 0:HL], AF.Exp,
                                 accum_out=z1[:])
            nc.tensor.matmul(ab_ps[:, HL:L], wu_sb[0:1, 0:128],
                             ex[0:1, HL:L], start=True, stop=True,
                             skip_group_check=True)
            nc.vector.tensor_tensor(zs[:], z0[:], z1[:], ALU.add)
            nc.vector.reciprocal(rz[:], zs[:])
            nc.gpsimd.partition_broadcast(rzb[:], rz[:])
            for h in range(HC):
                trash = smp.tile([128, L], F16, tag="trash", name=f"tr1{h}")
                nc.vector.scalar_tensor_tensor(
                    trash[:, 0:HL], xt16_sb[:, s * HC + h, HL:L], 1.0,
                    ab_ps[:, HL:L], ALU.mult, ALU.mult,
                    accum_out=pu2[:, HC + h:HC + h + 1],
                )
            # combine: pooled = (pu_half0 + pu_half1) * rz, then drain
            pusum = smp.tile([128, HC], F32, tag="pusum")
            nc.vector.tensor_tensor(pusum[:], pu2[:, 0:HC], pu2[:, HC:2 * HC],
                                    ALU.add)
            nc.vector.tensor_scalar(pucat[:, s * HC:(s + 1) * HC], pusum[:],
                                    rzb[:, 0:1], None, ALU.mult)
            tp1 = tps.tile([HC, 128], F32, tag="tp", name="tp1")
            nc.tensor.transpose(tp1[:], pucat[:, s * HC:(s + 1) * HC],
                                ident[:])
            orow1 = outp.tile([HC, 128], F32, tag="orow1")
            nc.vector.tensor_copy(orow1[:], tp1[:])
            nc.sync.dma_start(out_r[(SPC - 1) * HC:SPC * HC, :], orow1[:])

    nc.finalize()
    return nc


def kernel(hidden_states, W_fc, b_fc, W_nu, _trace=False, _trace_kwargs=None):
    from concourse.bass_utils import run_bass_kernel_spmd

    E4 = ml_dtypes.float8_e4m3

    hs = np.ascontiguousarray(hidden_states, dtype=np.float32)
    W_fc = np.asarray(W_fc, np.float32)
    b_fc = np.asarray(b_fc, np.float32)
    W_nu = np.asarray(W_nu, np.float32)

    # permute the FC output dim so |W_nu| is ascending; fp8 noise then
    # lands on the low-|W_nu| output chunks
    perm = np.argsort(np.abs(W_nu), kind="stable")
    Wp = W_fc[perm]
    bp = b_fc[perm]
    wnup = W_nu[perm]

    # W^T chunk layouts: wth[p, kout, cin, m] = Wp[kout*128+m, cin*128+p]
    wth = Wp.T.reshape(HC, 128, HC, 128).transpose(1, 2, 0, 3)
    wt8_host = np.ascontiguousarray(
        wth[:, 0:NK8, :, :]                              # [p, k, cin, m]
        .reshape(128, NK8, NP, 2, 128)                   # pair cin chunks
        .reshape(128, NK8 * NP, 2, 128).astype(E4))
    wt16_host = np.ascontiguousarray(wth[:, HC - 1, :, :].astype(np.float16))
    bfc_host = np.ascontiguousarray(bp.reshape(HC, 128).T, np.float32)
    wnu_host = np.ascontiguousarray(wnup.reshape(HC, 128).T.astype(np.float16))

    in_maps = []
    for c in range(NCORES):
        # X^T in sample-major [128, (s c t)] layout so each per-sample DMA
        # is 128 contiguous descriptors: [p, s, c, t] = X[s*512+t, c*128+p]
        xt = np.ascontiguousarray(
            hs[c * SPC:(c + 1) * SPC].reshape(TOK, H).T
            .reshape(HC, 128, SPC, L).transpose(1, 2, 0, 3)
            .reshape(128, SPC * HC * L))
        in_maps.append({
            "xt8": xt.astype(E4),
            "xt16": xt.astype(np.float16),
            "wt8": wt8_host, "wt16": wt16_host,
            "bfc": bfc_host, "wnu": wnu_host,
        })

    if "nc" not in _compiled:
        _compiled["nc"] = _build()
    res = run_bass_kernel_spmd(
        _compiled["nc"], in_maps, list(range(NCORES)),
        trace=_trace, **(_trace_kwargs or {}),
    )
    kernel.last_results = res
    out = np.concatenate([np.asarray(r["out"], np.float32) for r in res.results])
    return out


# revision 20
# speedup vs baseline: 1.3106x; 1.0421x over previous
"""AttentionProtoNet pooling kernel for 8x TRN2 NeuronCores.

reference (per sample of B=64, L=512, H=768):
    upsilon = tanh(hs @ W_fc.T + b_fc)        [L, H]
    nu      = upsilon @ W_nu                  [L]
    alphas  = softmax(nu)                     [L]
    pooled  = alphas @ hs                     [H]

Strategy: data-parallel over B (8 samples per core).

Mixed-precision via |W_nu|-sorting: the FC output dim is permuted on the
host so |W_nu| is ascending.  nu-noise from quantization scales with the
W_nu^2 mass of the chunks quantized, so the low-mass output chunks run
cheap and the heavy chunk runs precise:
  - output chunks 0..4 (42% of W_nu^2 mass): fp8e4 DoubleRow matmuls,
    2 contraction chunks per MM -> 3 MMs instead of 6 per output chunk;
  - output chunk 5 (58% of the mass): fp16, 6 MMs;
  - ups chunks 0..3 (19% of the mass) are stored fp8 and their nu
    contribution uses 2 DoubleRow MMs instead of 4 fp16 MMs;
  - pooling always reads the fp16 X copy.
End-to-end rel err ~1.75e-2 (gate 2e-2), measured bit-stable run-to-run.

DMA: ONE HWDGE queue (a second queue round-robins packets at the SDMA
level and starves the small weight transfers behind the X flood), with
weights first, then per-sample X with fp8 leading fp16 by one sample.
Each sample's fp16 output chunk + epilogue are deferred into the next
sample's fp8 stream so they never wait on the slower xt16 DMA.  The last
sample runs in two 256-token halves so its softmax/pooling chain
overlaps its own matmuls; its exp-broadcast rides a K=1 ones-matmul into
PSUM instead of the slower gpsimd broadcast.  Samples 0..5 drain
(transpose + copy + store) during the last sample's stream.
"""

import sys

sys.path.insert(0, "/opt/trn_rl_repo")

import numpy as np
import ml_dtypes

B, L, H = 64, 512, 768
NCORES = 8
SPC = B // NCORES            # samples per core
TOK = SPC * L                # tokens per core
HC = H // 128                # 128-partition chunks of H
NK8 = 5                      # fp8 output chunks (sorted by |wnu| asc)
NU8 = 4                      # ups chunks stored fp8 for the nu matmul
NP = HC // 2                 # contraction pairs for DoubleRow
HL = L // 2                  # token half for the last sample
WARMUP_MM = 14               # junk matmuls: p-state + HAM ramp during DMA

_compiled = {}


def _build():
    import concourse.bass as bass
    import concourse.bacc as bacc
    import concourse.tile as tile
    from concourse import mybir
    from concourse.masks import make_identity

    F32 = mybir.dt.float32
    F16 = mybir.dt.float16
    F8 = mybir.dt.float8e4
    AF = mybir.ActivationFunctionType
    ALU = mybir.AluOpType
    DR = mybir.MatmulPerfMode.DoubleRow

    nc = bacc.Bacc(None, target_bir_lowering=False)

    xt8_d = nc.dram_tensor("xt8", [128, SPC * HC * L], F8, kind="ExternalInput")
    xt16_d = nc.dram_tensor("xt16", [128, SPC * HC * L], F16,
                            kind="ExternalInput")
    # fp8 weights for kouts 0..4: [p, k*3+j, i, m] = Wp[k*128+m, (2j+i)*128+p]
    wt8_d = nc.dram_tensor("wt8", [128, NK8 * NP, 2, 128], F8,
                           kind="ExternalInput")
    # fp16 weights for kout 5: [p, h, m] = Wp[5*128+m, h*128+p]
    wt16_d = nc.dram_tensor("wt16", [128, HC, 128], F16, kind="ExternalInput")
    bfc_d = nc.dram_tensor("bfc", [128, HC], F32, kind="ExternalInput")
    wnu_d = nc.dram_tensor("wnu", [128, HC], F16, kind="ExternalInput")
    # fp8 wnu pairs for the DR nu matmul: [p, j, i, 0] = wnu[(2j+i)*128+p]
    wnu8_d = nc.dram_tensor("wnu8", [128, NU8 // 2, 2, 16], F8,
                            kind="ExternalInput")
    out_d = nc.dram_tensor("out", [SPC, H], F32, kind="ExternalOutput")

    with tile.TileContext(nc) as tc:
        with tc.tile_pool(name="xp", bufs=1) as xp, \
             tc.tile_pool(name="wp", bufs=1) as wp, \
             tc.tile_pool(name="cst", bufs=1) as cst, \
             tc.tile_pool(name="ups", bufs=2) as upsp, \
             tc.tile_pool(name="sm", bufs=2) as smp, \
             tc.tile_pool(name="outp", bufs=2) as outp, \
             tc.tile_pool(name="mmps", bufs=3, space="PSUM") as mmps, \
             tc.tile_pool(name="nups", bufs=2, space="PSUM") as nups, \
             tc.tile_pool(name="abp", bufs=1, space="PSUM") as abp, \
             tc.tile_pool(name="tps", bufs=2, space="PSUM") as tps:

            # ---- PE warmup: junk matmuls with no DMA dependency ramp the
            # p-state and the HAM activity window while the first tiles
            # stream in.
            wu_sb = cst.tile([128, L], F16)
            nc.vector.memset(wu_sb[:], 1.0)
            wu_ps = tps.tile([128, L], F32, tag="tp", name="wu_ps")
            for i in range(WARMUP_MM):
                nc.tensor.matmul(wu_ps[:], wu_sb[:, 0:128], wu_sb[:],
                                 start=(i == 0), stop=(i == WARMUP_MM - 1))

            # ---- SBUF tiles
            bfc_sb = cst.tile([128, HC], F32)
            wnu_sb = cst.tile([128, HC], F16)
            wnu8_sb = cst.tile([128, NU8 // 2, 2, 16], F8)
            wt8_sb = wp.tile([128, NK8 * NP, 2, 128], F8)
            wt16_sb = wp.tile([128, HC, 128], F16)
            xt8_sb = xp.tile([128, SPC * HC, L], F8)
            xt16_sb = xp.tile([128, SPC * HC, L], F16)
            ident = cst.tile([128, 128], F32)

            # ---- DMA: ONE HWDGE queue, weights strictly first
            nc.sync.dma_start(wt8_sb[:, 0:NP], wt8_d[:, 0:NP])
            nc.sync.dma_start(bfc_sb[:], bfc_d[:])
            nc.sync.dma_start(wnu_sb[:], wnu_d[:])
            nc.sync.dma_start(wnu8_sb[:], wnu8_d[:])
            nc.sync.dma_start(wt16_sb[:], wt16_d[:])
            nc.sync.dma_start(wt8_sb[:, NP:NK8 * NP], wt8_d[:, NP:NK8 * NP])

            def dma_xt(sb, d, s, c0, c1):
                nc.sync.dma_start(
                    sb[:, s * HC + c0:s * HC + c1, :],
                    d[:, (s * HC + c0) * L:(s * HC + c1) * L].rearrange(
                        "p (c t) -> p c t", t=L),
                )
            # xt8 leads xt16 by one sample: sample s's fp16 work (kout5 +
            # pooling + epilogue) runs deferred into sample s+1's stream
            dma_xt(xt8_sb, xt8_d, 0, 0, 2)
            dma_xt(xt8_sb, xt8_d, 0, 2, HC)
            dma_xt(xt8_sb, xt8_d, 1, 0, HC)
            for s in range(2, SPC):
                dma_xt(xt16_sb, xt16_d, s - 2, 0, HC)
                dma_xt(xt8_sb, xt8_d, s, 0, HC)
            dma_xt(xt16_sb, xt16_d, SPC - 2, 0, HC)
            dma_xt(xt16_sb, xt16_d, SPC - 1, 0, HC)
            make_identity(nc, ident[:])

            # ---- per-sample state carried to later emission points
            ups8_t = [None] * SPC
            ups16_t = [None] * SPC
            # all samples' pooled vectors gather into one [128, 48] tile
            pucat = outp.tile([128, SPC * HC], F32, tag="pucat")

            def emit_group8(s, k, ups8, ups16, c0, c1):
                """fp8 DoubleRow matmul group + tanh, kout k, tokens [c0,c1).
                tanh output goes to fp8 ups for k < NU8, fp16 above."""
                ps = mmps.tile([128, L], F32, tag="mm")
                w = c1 - c0
                for j in range(NP):
                    nc.tensor.matmul(
                        ps[:, 0:w],
                        wt8_sb[:, k * NP + j],
                        xt8_sb[:, s * HC + 2 * j:s * HC + 2 * j + 2, c0:c1],
                        start=(j == 0),
                        stop=(j == NP - 1),
                        perf_mode=DR,
                    )
                dst = ups8[:, k, c0:c1] if k < NU8 else ups16[:, 0, c0:c1]
                nc.scalar.activation(dst, ps[:, 0:w], AF.Tanh,
                                     bias=bfc_sb[:, k:k + 1])

            def emit_group16(s, ups16, c0, c1):
                """fp16 matmul group + tanh for kout 5, tokens [c0, c1)."""
                ps = mmps.tile([128, L], F32, tag="mm")
                w = c1 - c0
                for h in range(HC):
                    nc.tensor.matmul(
                        ps[:, 0:w],
                        wt16_sb[:, h],
                        xt16_sb[:, s * HC + h, c0:c1],
                        start=(h == 0),
                        stop=(h == HC - 1),
                    )
                nc.scalar.activation(ups16[:, 1, c0:c1], ps[:, 0:w], AF.Tanh,
                                     bias=bfc_sb[:, HC - 1:HC])

            def emit_nu(nu, s, c0, c1, start, stop, which):
                """one nu partial-accumulation MM; which in 0..3:
                0,1 = DR pairs (chunks 0,1 / 2,3); 2,3 = fp16 chunks 4,5."""
                if which < NU8 // 2:
                    j = which
                    nc.tensor.matmul(
                        nu[:, 0:c1 - c0], wnu8_sb[:, j, :, 0:1],
                        ups8_t[s][:, 2 * j:2 * j + 2, c0:c1],
                        start=start, stop=stop, perf_mode=DR,
                    )
                else:
                    k = NU8 + which - 2
                    nc.tensor.matmul(
                        nu[:, 0:c1 - c0], wnu_sb[:, k:k + 1],
                        ups16_t[s][:, k - NU8, c0:c1],
                        start=start, stop=stop,
                    )

            def emit_epilogue(s):
                """nu + softmax + pooling for sample s, emitted inside a
                later sample's matmul stream where all inputs are done."""
                nu = nups.tile([1, L], F32, tag="nu", name=f"nu_{s}")
                for w in range(4):
                    emit_nu(nu, s, 0, L, w == 0, w == 3, w)
                # nu is small enough that exp() needs no max subtraction
                ex = smp.tile([1, L], F16, tag="ex")
                z = smp.tile([1, 1], F32, tag="z")
                rz = smp.tile([1, 1], F32, tag="rz")
                nc.scalar.activation(ex[:], nu[:], AF.Exp, accum_out=z[:])
                nc.vector.reciprocal(rz[:], z[:])
                ab = smp.tile([128, L], F16, tag="ab")
                nc.gpsimd.partition_broadcast(ab[:], ex[:])
                rzb = smp.tile([128, 1], F32, tag="rzb")
                nc.gpsimd.partition_broadcast(rzb[:], rz[:])
                # weighted-sum pooling on the VectorEngine; the 1/Z
                # normalization rides the STT per-partition scalar
                for h in range(HC):
                    trash = smp.tile([128, L], F16, tag="trash")
                    nc.vector.scalar_tensor_tensor(
                        trash[:], xt16_sb[:, s * HC + h, :], rzb[:, 0:1],
                        ab[:], ALU.mult, ALU.mult,
                        accum_out=pucat[:, s * HC + h:s * HC + h + 1],
                    )

            # ---- samples 0..6: full-width pipeline.  Sample s's fp16
            # group (kout 5, needs xt16[s]) and its epilogue are deferred
            # into sample s+1's fp8 stream so neither waits on xt16 DMA.
            for s in range(SPC - 1):
                ups8_t[s] = upsp.tile([128, NU8, L], F8, tag="ups8",
                                      name=f"u8_{s}")
                ups16_t[s] = upsp.tile([128, 2, L], F16, tag="ups16",
                                       name=f"u16_{s}")
                for k in range(NK8):
                    emit_group8(s, k, ups8_t[s], ups16_t[s], 0, L)
                    if s > 0 and k == 1:
                        emit_group16(s - 1, ups16_t[s - 1], 0, L)
                    if s > 0 and k == 3:
                        emit_epilogue(s - 1)

            # ---- last sample: two 256-token halves so the softmax/pool
            # chain of half 0 overlaps half 1's matmuls, and only a short
            # chain trails the final matmul
            s = SPC - 1
            ups8_t[s] = upsp.tile([128, NU8, L], F8, tag="ups8", name="u8l")
            ups16_t[s] = upsp.tile([128, 2, L], F16, tag="ups16", name="u16l")

            def emit_group(s, k, c0, c1):
                if k < NK8:
                    emit_group8(s, k, ups8_t[s], ups16_t[s], c0, c1)
                else:
                    emit_group16(s, ups16_t[s], c0, c1)

            nu_a = nups.tile([1, L], F32, tag="nu", name="nu_a")
            nu_b = None
            ex = smp.tile([1, L], F16, tag="ex", name="ex_l")
            ab_ps = abp.tile([128, L], F32, tag="abps", name="ab_ps")
            z0 = smp.tile([1, 1], F32, tag="z", name="z0")
            z1 = smp.tile([1, 1], F32, tag="z", name="z1")
            pu2 = outp.tile([128, 2 * HC], F32, tag="pu2")

            # half 0 (nu partials trail their chunks' tanh)
            for ji, k in enumerate(range(HC)):
                emit_group(s, k, 0, HL)
                if ji == 0:
                    emit_group16(s - 1, ups16_t[s - 1], 0, L)
                if ji == 1:
                    emit_epilogue(s - 1)
                if ji == 2:
                    emit_nu(nu_a, s, 0, HL, True, False, 0)
                if ji == 4:
                    emit_nu(nu_a, s, 0, HL, False, False, 1)
            # half 1 (k=5 first so tanh(k=4) barely gates the nu tail)
            for ji, k in enumerate([5, 0, 1, 2, 3, 4]):
                emit_group(s, k, HL, L)
                if ji == 0:
                    emit_nu(nu_a, s, 0, HL, False, False, 3)
                if ji == 1:
                    emit_nu(nu_a, s, 0, HL, False, True, 2)
                    # half-0 epilogue: exp broadcast via a K=1 ones-matmul
                    # into PSUM; unnormalized pooling into pu2[:, 0:6]
                    nc.scalar.activation(ex[:, 0:HL], nu_a[:, 0:HL], AF.Exp,
                                         accum_out=z0[:])
                    nc.tensor.matmul(ab_ps[:, 0:HL], wu_sb[0:1, 0:128],
                                     ex[0:1, 0:HL], start=True, stop=True,
                                     skip_group_check=True)
                    for h in range(HC):
                        trash = smp.tile([128, L], F16, tag="trash",
                                         name=f"tr0{h}")
                        nc.vector.scalar_tensor_tensor(
                            trash[:, 0:HL], xt16_sb[:, s * HC + h, 0:HL], 1.0,
                            ab_ps[:, 0:HL], ALU.mult, ALU.mult,
                            accum_out=pu2[:, h:h + 1],
                        )
                if ji == 3:
                    if nu_b is None:
                        nu_b = nups.tile([1, L], F32, tag="nu", name="nu_b")
                    emit_nu(nu_b, s, HL, L, True, False, 0)
                if ji == 5:
                    emit_nu(nu_b, s, HL, L, False, False, 1)
            emit_nu(nu_b, s, HL, L, False, False, 3)
            emit_nu(nu_b, s, HL, L, False, True, 2)

            # samples 0..5 drain early, during the last sample's tail: the
            # PE transpose sits here in queue order so its deps (pooling of
            # samples <= 5) are long satisfied and it never blocks the queue
            tp0 = tps.tile([(SPC - 2) * HC, 128], F32, tag="tp", name="tp0")
            nc.tensor.transpose(tp0[:], pucat[:, 0:(SPC - 2) * HC], ident[:])
            orow0 = outp.tile([(SPC - 2) * HC, 128], F32, tag="orow0")
            nc.vector.tensor_copy(orow0[:], tp0[:])
            out_r = out_d[:, :].rearrange("s (c p) -> (s c) p", p=128)
            nc.sync.dma_start(out_r[0:(SPC - 2) * HC, :], orow0[:])

            # half-1 epilogue: unnormalized pooling into pu2[:, 6:12];
            # normalization happens once on the tiny [128, 6] combine
            zs = smp.tile([1, 1], F32, tag="zs")
            rz = smp.tile([1, 1], F32, tag="rz", name="rz_l")
            rzb = smp.tile([128, 1], F32, tag="rzb", name="rzb_l")
            nc.scalar.activation(ex[:, HL:L], nu_b[:, 0:HL], AF.Exp,
                                 accum_out=z1[:])
            nc.tensor.matmul(ab_ps[:, HL:L], wu_sb[0:1, 0:128],
                             ex[0:1, HL:L], start=True, stop=True,
                             skip_group_check=True)
            nc.vector.tensor_tensor(zs[:], z0[:], z1[:], ALU.add)
            nc.vector.reciprocal(rz[:], zs[:])
            nc.gpsimd.partition_broadcast(rzb[:], rz[:])
            for h in range(HC):
                trash = smp.tile([128, L], F16, tag="trash", name=f"tr1{h}")
                nc.vector.scalar_tensor_tensor(
                    trash[:, 0:HL], xt16_sb[:, s * HC + h, HL:L], 1.0,
                    ab_ps[:, HL:L], ALU.mult, ALU.mult,
                    accum_out=pu2[:, HC + h:HC + h + 1],
                )
            # combine: pooled = (pu_half0 + pu_half1) * rz, then drain
            # samples 6..7 together
            pusum = smp.tile([128, HC], F32, tag="pusum")
            nc.vector.tensor_tensor(pusum[:], pu2[:, 0:HC], pu2[:, HC:2 * HC],
                                    ALU.add)
            nc.vector.tensor_scalar(pucat[:, s * HC:(s + 1) * HC], pusum[:],
                                    rzb[:, 0:1], None, ALU.mult)
            tp1 = tps.tile([2 * HC, 128], F32, tag="tp", name="tp1")
            nc.tensor.transpose(tp1[:], pucat[:, (SPC - 2) * HC:SPC * HC],
                                ident[:])
            orow1 = outp.tile([2 * HC, 128], F32, tag="orow1")
            nc.vector.tensor_copy(orow1[:], tp1[:])
            nc.sync.dma_start(out_r[(SPC - 2) * HC:SPC * HC, :], orow1[:])

    nc.finalize()
    return nc


def kernel(hidden_states, W_fc, b_fc, W_nu, _trace=False, _trace_kwargs=None):
    from concourse.bass_utils import run_bass_kernel_spmd

    E4 = ml_dtypes.float8_e4m3

    hs = np.ascontiguousarray(hidden_states, dtype=np.float32)
    W_fc = np.asarray(W_fc, np.float32)
    b_fc = np.asarray(b_fc, np.float32)
    W_nu = np.asarray(W_nu, np.float32)

    # permute the FC output dim so |W_nu| is ascending; quantization noise
    # then lands on the low-|W_nu| output chunks
    perm = np.argsort(np.abs(W_nu), kind="stable")
    Wp = W_fc[perm]
    bp = b_fc[perm]
    wnup = W_nu[perm]

    # W^T chunk layouts: wth[p, kout, cin, m] = Wp[kout*128+m, cin*128+p]
    wth = Wp.T.reshape(HC, 128, HC, 128).transpose(1, 2, 0, 3)
    wt8_host = np.ascontiguousarray(
        wth[:, 0:NK8, :, :]                              # [p, k, cin, m]
        .reshape(128, NK8, NP, 2, 128)                   # pair cin chunks
        .reshape(128, NK8 * NP, 2, 128).astype(E4))
    wt16_host = np.ascontiguousarray(wth[:, HC - 1, :, :].astype(np.float16))
    bfc_host = np.ascontiguousarray(bp.reshape(HC, 128).T, np.float32)
    wnu_host = np.ascontiguousarray(wnup.reshape(HC, 128).T.astype(np.float16))
    wnu8_host = np.zeros((128, NU8 // 2, 2, 16), E4)
    wnu8_host[:, :, :, 0] = (wnup[0:NU8 * 128].reshape(NU8 // 2, 2, 128)
                             .transpose(2, 0, 1).astype(E4))

    in_maps = []
    for c in range(NCORES):
        # X^T in sample-major [128, (s c t)] layout so each per-sample DMA
        # is 128 contiguous descriptors: [p, s, c, t] = X[s*512+t, c*128+p]
        xt = np.ascontiguousarray(
            hs[c * SPC:(c + 1) * SPC].reshape(TOK, H).T
            .reshape(HC, 128, SPC, L).transpose(1, 2, 0, 3)
            .reshape(128, SPC * HC * L))
        in_maps.append({
            "xt8": xt.astype(E4),
            "xt16": xt.astype(np.float16),
            "wt8": wt8_host, "wt16": wt16_host,
            "bfc": bfc_host, "wnu": wnu_host, "wnu8": wnu8_host,
        })

    if "nc" not in _compiled:
        _compiled["nc"] = _build()
    res = run_bass_kernel_spmd(
        _compiled["nc"], in_maps, list(range(NCORES)),
        trace=_trace, **(_trace_kwargs or {}),
    )
    kernel.last_results = res
    out = np.concatenate([np.asarray(r["out"], np.float32) for r in res.results])
    return out
